# revision 11
# baseline (speedup 1.0000x reference)
"""Trainium2 Bass kernel for nn_CCL__69277822485245 (spectral conv via DCT/FFT).

Math: the reference's rFFT along W cancels into a circular 5-tap convolution,
and the DCT-II sandwich M @ diag(D[:,s]) @ D collapses into 5 dense 128x128
matrices G_s (precomputed on host). Per batch element:

    u_s[i, m, w] = sum_h G_s[m, h] x[i, h, w]                  (stage 1)
    out[o, m, n] = sum_{s,t,i} W[o,i,s,t] u_s[i, m, (n-t)%W] + bias[o]   (stage 2)

Sharding: data-parallel over batch B=8 across the 8 NeuronCores (1 each).

v3 design (from the v1/v2 traces):
  - x resident in SBUF as [h=128, (w128, di128)]: stage-1 lhsT = x[:, w, :]
    is contiguous -> fast LDWEIGHTS.  di duplicates i on both partition
    halves (d = s-parity selector).
  - stage 1 per m-half (mh), full W (no halo recompute): one matmul per w
    (N=320 = (sidx5, m64), s-order [0,2,4,1,3]), TWO w per psum tile so
    each psum->SBUF cast moves 384/256 elements (v2's 192/128-el casts
    were fixed-overhead dominated).  Top partitions keep s={0,2,4}
    (Vector engine), bottom keep s={1,3} (Scalar engine) - contiguous
    64-el inner runs.
  - u lives in per-32-w-block "window" tiles [128, (c3, slot36, m64)]:
    slot j holds w = blk*32 - 4 + j, so stage-2's shifted reads never
    wrap.  Window-boundary slots are seeded by bf16 SBUF->SBUF GpSimd
    copies (window k slots 32:36 -> window k+1 slots 0:4).
  - stage 2 per (blk, jt-pair): psum [128, (jt2, j8, m64)=1024], 30
    accumulation passes (jt2 x t5 x c3), K=128 (c<2) / K=64 (c=2), rhs =
    contiguous (j8, m64)=512 run of the window tile.  One 1024-el
    Vector-engine tensor_scalar_add evacuates psum + bias into oacc
    (transposed to (m, w)).
  - emission interleaves stage-1 units 1:2 into stage-2 matmuls of the
    previous block so the in-order PE never waits on casts.
  - x is DMA'd in 5 w-chunks in consumption order; oacc per mh is
    DMA'd while the other mh computes.
"""

import numpy as np

H = 128
W = 128
CI = 64
CO = 128
KH = 5
KW = 5
B = 8

MH = 64          # m-half processed per outer pass
WB = 32          # w-block (stage-2 granularity)
HALO = 4         # extra leading slots per window for the t-shifts
NSLOT = 3        # s-slots per partition half (s = 2c + d)
WIN = WB + HALO  # 36 slots per window tile
NBLK = W // WB   # 4
JT = 8           # j-extent per stage-2 accumulation region (N = JT*MH = 512)

_PROG = None
_CONSTS = None
_RUN_OPTS = {}     # test harness may set e.g. {"trace": True, "trace_cores": [0]}
_LAST_RESULT = None


def _np_dt():
    import ml_dtypes
    return ml_dtypes.bfloat16


def _build_consts():
    n = np.arange(H, dtype=np.float64)
    ang = np.pi * (2.0 * n[None, :] + 1.0) * n[:, None] / (2.0 * H)  # [k, h]
    D = 2.0 * np.cos(ang)
    wgt = np.where(n == 0, 0.5, 1.0)
    M = (np.cos(ang).T * wgt[None, :]) / (2.0 * H)                    # [m, k]
    G = np.stack([M @ (D[:, s:s + 1] * D) for s in range(KH)])        # [s, m, h]
    G = G[[0, 2, 4, 1, 3]]   # s-order so each half's psum->u copy is contiguous
    # rhs layout [h, (mh, sidx, ml)]: col = mh*320 + sidx*64 + ml
    GT = (G.transpose(2, 0, 1)                # [h, s, m]
            .reshape(H, KH, 2, MH)            # [h, s, mh, ml]
            .transpose(0, 2, 1, 3)            # [h, mh, s, ml]
            .reshape(H, KH * H))
    return np.ascontiguousarray(GT).astype(_np_dt())


# x DMA chunks (w ranges), in stage-1 consumption order
_XCHUNKS = [(124, 128), (0, 32), (32, 64), (64, 96), (96, 124)]


def _build_program():
    import concourse.mybir as mybir
    import concourse.tile as tile
    from concourse import bacc

    f32 = mybir.dt.float32
    bf16 = mybir.dt.bfloat16

    nc = bacc.Bacc("TRN2", target_bir_lowering=False, debug=False,
                   enable_asserts=False, num_devices=B)
    x_d = nc.dram_tensor("x", [H, W * 2 * CI], bf16, kind="ExternalInput").ap()
    g_d = nc.dram_tensor("g", [H, KH * H], bf16, kind="ExternalInput").ap()
    w_d = nc.dram_tensor("wt", [128, KW * NSLOT * CO], bf16,
                         kind="ExternalInput").ap()
    b_d = nc.dram_tensor("bias", [CO, 1], f32, kind="ExternalInput").ap()
    o_d = nc.dram_tensor("out", [CO, H, W], f32, kind="ExternalOutput").ap()

    with tile.TileContext(nc) as tc:
        with (
            tc.tile_pool(name="const", bufs=1) as cpool,
            tc.tile_pool(name="win", bufs=3) as wpool,
            tc.tile_pool(name="oacc", bufs=1) as opool,
            tc.tile_pool(name="ps1", bufs=2, space="PSUM") as ps1,
            tc.tile_pool(name="ps2", bufs=2, space="PSUM") as ps2,
        ):
            gt = cpool.tile([H, KH * H], bf16)
            nc.sync.dma_start(gt[:], g_d)
            # x in w-chunks so early stage-1 work doesn't wait on all of x
            xts = {}
            for (w0, w1) in _XCHUNKS:
                xt = cpool.tile([H, (w1 - w0) * 2 * CI], bf16,
                                tag=f"x{w0}")
                nc.sync.dma_start(xt[:], x_d[:, w0 * 2 * CI:w1 * 2 * CI])
                xts[(w0, w1)] = xt[:].rearrange("p (w di) -> p w di",
                                                di=2 * CI)
            wt = cpool.tile([128, KW * NSLOT * CO], bf16)
            nc.sync.dma_start(wt[:], w_d)
            bt = cpool.tile([CO, 1], f32)
            nc.sync.dma_start(bt[:], b_d)

            def xcol(w):
                for (w0, w1) in _XCHUNKS:
                    if w0 <= w < w1:
                        return xts[(w0, w1)][:, w - w0, :]
                raise AssertionError(w)

            # ---------------- emission units ----------------

            def s1_pair_unit(mh, wa, wb, wtile, slot):
                """Two stage-1 columns sharing a psum tile + 2 batched casts."""
                def emit():
                    # two 512-col bank-aligned regions: a matmul's output
                    # must not cross a PSUM bank boundary
                    p1 = ps1.tile([128, 1024], f32, name="p1")
                    for dj, w in enumerate((wa, wb)):
                        nc.tensor.matmul(
                            p1[:, dj * 512:dj * 512 + KH * MH], xcol(w),
                            gt[:, mh * KH * MH:(mh + 1) * KH * MH],
                            start=True, stop=True)
                    w3 = wtile[:].rearrange("p (c j m) -> p c j m",
                                            c=NSLOT, j=WIN)
                    p4 = p1[:].rearrange("p (j q) -> p j q", j=2)
                    ptop = p4[0:64, :, 0:NSLOT * MH].rearrange(
                        "p j (c m) -> p j c m", c=NSLOT)
                    pbot = p4[64:128, :, NSLOT * MH:KH * MH].rearrange(
                        "p j (c m) -> p j c m", c=2)
                    # top half keeps s={0,2,4} -> slots c=0..2
                    nc.vector.tensor_copy(
                        w3[0:64, :, slot:slot + 2, :].transpose([0, 2, 1, 3]),
                        ptop)
                    # bottom half keeps s={1,3} -> slots c=0..1
                    nc.scalar.copy(
                        w3[64:128, 0:2, slot:slot + 2, :]
                        .transpose([0, 2, 1, 3]),
                        pbot)
                return emit

            def dup_unit(wtile, wtile_next):
                """Seed window k+1 slots 0..3 from window k slots 32..35
                (bf16 SBUF->SBUF on GpSimd; psum is off-limits there)."""
                def emit():
                    w3 = wtile[:].rearrange("p (c j m) -> p c j m",
                                            c=NSLOT, j=WIN)
                    n3 = wtile_next[:].rearrange("p (c j m) -> p c j m",
                                                 c=NSLOT, j=WIN)
                    nc.gpsimd.tensor_copy(n3[:, 0:2, 0:HALO, :],
                                          w3[:, 0:2, WB:WIN, :])
                    nc.gpsimd.tensor_copy(n3[0:64, 2, 0:HALO, :],
                                          w3[0:64, 2, WB:WIN, :])
                return emit

            def s2_mm_unit(wtile, blk, jp, half, t, c, p2holder):
                def emit():
                    if p2holder[0] is None:
                        p2holder[0] = ps2.tile([128, 2 * JT * MH], f32,
                                               name="p2")
                    p2 = p2holder[0]
                    jt = 2 * jp + half
                    kk = 128 if c < 2 else 64
                    lhsT2 = wt[0:kk, (t * NSLOT + c) * CO:
                               (t * NSLOT + c + 1) * CO]
                    w3 = wtile[:].rearrange("p (c j m) -> p c j m",
                                            c=NSLOT, j=WIN)
                    rhs2 = w3[0:kk, c, jt * JT - t + HALO:
                              (jt + 1) * JT - t + HALO, :]
                    start = (t == 0 and c == 0)
                    stop = (t == KW - 1 and c == NSLOT - 1)
                    nc.tensor.matmul(p2[:, half * JT * MH:
                                        (half + 1) * JT * MH],
                                     lhsT2, rhs2, start=start, stop=stop)
                return emit

            def s2_evac_unit(mh, blk, jp, oa, p2holder):
                def emit():
                    p2 = p2holder[0]
                    p23 = p2[:].rearrange("p (j m) -> p j m", j=2 * JT)
                    oa3 = oa[:].rearrange("p (m w) -> p m w", w=W)
                    nc.vector.tensor_scalar_add(
                        oa3[:, :, blk * WB + jp * 2 * JT:
                            blk * WB + (jp + 1) * 2 * JT],
                        p23[:].transpose([0, 2, 1]), bt[:])
                    p2holder[0] = None
                return emit

            # ---------------- schedule ----------------

            def s1_window_units(mh, k, wtile, wtile_next):
                units = []
                if k == 0:
                    fresh = list(range(124, 128)) + list(range(0, 32))
                else:
                    fresh = list(range(32 * k, 32 * k + 32))
                for i in range(0, len(fresh), 2):
                    wa, wb = fresh[i], fresh[i + 1]
                    if k == 0 and wa >= 124:
                        slot = wa - 124
                    else:
                        slot = wa - (32 * k - 4)
                    units.append(s1_pair_unit(mh, wa, wb, wtile, slot))
                if k < NBLK - 1:
                    units.append(dup_unit(wtile, wtile_next))
                return units

            def s2_block_units(mh, k, wtile, oa):
                units = []
                for jp in range(WB // (2 * JT)):
                    holder = [None]
                    for half in range(2):
                        for t in range(KW):
                            for c in range(NSLOT):
                                units.append(s2_mm_unit(wtile, k, jp, half,
                                                        t, c, holder))
                    units.append(s2_evac_unit(mh, k, jp, oa, holder))
                return units

            # build the full interleaved program
            oaccs = {}
            wtiles = {}

            def get_wtile(mh, k):
                if (mh, k) not in wtiles:
                    wtiles[(mh, k)] = wpool.tile(
                        [128, NSLOT * WIN * MH], bf16, name="win")
                return wtiles[(mh, k)]

            wins = [(mh, k) for mh in range(2) for k in range(NBLK)]
            for mh in range(2):
                oaccs[mh] = opool.tile([CO, MH * W], f32, tag=f"oacc{mh}",
                                       name=f"oacc{mh}")

            # prologue: first window fully
            mh0, k0 = wins[0]
            t0 = get_wtile(mh0, k0)
            t1 = get_wtile(*wins[1])
            for u in s1_window_units(mh0, k0, t0, t1):
                u()

            for idx, (mh, k) in enumerate(wins):
                wtile = get_wtile(mh, k)
                s2u = s2_block_units(mh, k, wtile, oaccs[mh])
                if idx + 1 < len(wins):
                    nmh, nk = wins[idx + 1]
                    ntile = get_wtile(nmh, nk)
                    nntile = (get_wtile(*wins[idx + 2])
                              if idx + 2 < len(wins) else None)
                    s1u = s1_window_units(nmh, nk, ntile, nntile)
                else:
                    s1u = []
                # interleave one s1 unit per two s2 units until exhausted
                si = 0
                for i, u in enumerate(s2u):
                    if i % 2 == 0 and si < len(s1u):
                        s1u[si]()
                        si += 1
                    u()
                for u in s1u[si:]:
                    u()
                if k == NBLK - 1:
                    nc.sync.dma_start(
                        o_d[:, mh * MH:(mh + 1) * MH, :],
                        oaccs[mh][:].rearrange("p (m w) -> p m w", w=W))
    nc.compile()
    return nc


def _get_prog():
    global _PROG
    if _PROG is None:
        _PROG = _build_program()
    return _PROG


def _build_wstack(weight):
    # wst[(d,i), (t, c, o)]: d=0 -> s=2c ; d=1 -> s=2c+1 (c<2), zeros for c=2
    wst = np.zeros((128, KW * NSLOT * CO), np.float32)
    for t in range(KW):
        for c in range(NSLOT):
            col = (t * NSLOT + c) * CO
            wst[0:64, col:col + CO] = weight[:, :, 2 * c, t].T
            if c < 2:
                wst[64:128, col:col + CO] = weight[:, :, 2 * c + 1, t].T
    return np.ascontiguousarray(wst).astype(_np_dt())


def kernel(x, weight, bias):
    from concourse.bass_utils import run_bass_kernel_spmd

    global _CONSTS
    if _CONSTS is None:
        _CONSTS = _build_consts()
    GT = _CONSTS

    x = np.ascontiguousarray(np.asarray(x, dtype=np.float32))
    weight = np.ascontiguousarray(np.asarray(weight, dtype=np.float32))
    bias = np.ascontiguousarray(np.asarray(bias, dtype=np.float32))

    wst = _build_wstack(weight)
    b2 = np.ascontiguousarray(bias.reshape(CO, 1))

    in_maps = []
    for b in range(B):
        # [h, (w, di)] with di = d*64 + i duplicated
        xt = x[b].transpose(1, 2, 0)                        # [H, W, ci]
        xdup = np.ascontiguousarray(
            np.concatenate([xt, xt], axis=2).reshape(H, W * 2 * CI)
        ).astype(_np_dt())
        in_maps.append({"x": xdup, "g": GT, "wt": wst, "bias": b2})

    res = run_bass_kernel_spmd(_get_prog(), in_maps, core_ids=list(range(B)),
                               **_RUN_OPTS)
    global _LAST_RESULT
    _LAST_RESULT = res
    out = np.stack([res.results[b]["out"] for b in range(B)], axis=0)
    return np.ascontiguousarray(out.astype(np.float32))


# revision 12
# speedup vs baseline: 1.2130x; 1.2130x over previous
"""Trainium2 Bass kernel for nn_CCL__69277822485245 (spectral conv via DCT/FFT).

Math: the reference's rFFT along W cancels into a circular 5-tap convolution,
and the DCT-II sandwich M @ diag(D[:,s]) @ D collapses into 5 dense 128x128
matrices G_s (precomputed on host). Per batch element:

    u_s[i, m, w] = sum_h G_s[m, h] x[i, h, w]                  (stage 1)
    out[o, m, n] = sum_{s,t,i} W[o,i,s,t] u_s[i, m, (n-t)%W] + bias[o]   (stage 2)

Sharding: data-parallel over batch B=8 across the 8 NeuronCores (1 each).

v3 design (from the v1/v2 traces):
  - x resident in SBUF as [h=128, (w128, di128)]: stage-1 lhsT = x[:, w, :]
    is contiguous -> fast LDWEIGHTS.  di duplicates i on both partition
    halves (d = s-parity selector).
  - stage 1 per m-half (mh), full W (no halo recompute): one matmul per w
    (N=320 = (sidx5, m64), s-order [0,2,4,1,3]), TWO w per psum tile so
    each psum->SBUF cast moves 384/256 elements (v2's 192/128-el casts
    were fixed-overhead dominated).  Top partitions keep s={0,2,4}
    (Vector engine), bottom keep s={1,3} (Scalar engine) - contiguous
    64-el inner runs.
  - u lives in per-32-w-block "window" tiles [128, (c3, slot36, m64)]:
    slot j holds w = blk*32 - 4 + j, so stage-2's shifted reads never
    wrap.  Window-boundary slots are seeded by bf16 SBUF->SBUF GpSimd
    copies (window k slots 32:36 -> window k+1 slots 0:4).
  - stage 2 per (blk, jt-pair): psum [128, (jt2, j8, m64)=1024], 30
    accumulation passes (jt2 x t5 x c3), K=128 (c<2) / K=64 (c=2), rhs =
    contiguous (j8, m64)=512 run of the window tile.  One 1024-el
    Vector-engine tensor_scalar_add evacuates psum + bias into oacc
    (transposed to (m, w)).
  - emission interleaves stage-1 units 1:2 into stage-2 matmuls of the
    previous block so the in-order PE never waits on casts.
  - x is DMA'd in 5 w-chunks in consumption order; oacc per mh is
    DMA'd while the other mh computes.
"""

import numpy as np

H = 128
W = 128
CI = 64
CO = 128
KH = 5
KW = 5
B = 8

MH = 64          # m-half processed per outer pass
WB = 32          # w-block (stage-2 granularity)
HALO = 4         # extra leading slots per window for the t-shifts
NSLOT = 3        # s-slots per partition half (s = 2c + d)
WIN = WB + HALO  # 36 slots per window tile
NBLK = W // WB   # 4
JT = 8           # j-extent per stage-2 accumulation region (N = JT*MH = 512)

_PROG = None
_CONSTS = None
_RUN_OPTS = {}     # test harness may set e.g. {"trace": True, "trace_cores": [0]}
_LAST_RESULT = None


def _np_dt():
    import ml_dtypes
    return ml_dtypes.bfloat16


def _build_consts():
    n = np.arange(H, dtype=np.float64)
    ang = np.pi * (2.0 * n[None, :] + 1.0) * n[:, None] / (2.0 * H)  # [k, h]
    D = 2.0 * np.cos(ang)
    wgt = np.where(n == 0, 0.5, 1.0)
    M = (np.cos(ang).T * wgt[None, :]) / (2.0 * H)                    # [m, k]
    G = np.stack([M @ (D[:, s:s + 1] * D) for s in range(KH)])        # [s, m, h]
    G = G[[0, 2, 4, 1, 3]]   # s-order so each half's psum->u copy is contiguous
    # rhs layout [h, (mh, sidx, ml)]: col = mh*320 + sidx*64 + ml
    GT = (G.transpose(2, 0, 1)                # [h, s, m]
            .reshape(H, KH, 2, MH)            # [h, s, mh, ml]
            .transpose(0, 2, 1, 3)            # [h, mh, s, ml]
            .reshape(H, KH * H))
    return np.ascontiguousarray(GT).astype(_np_dt())


# x DMA chunks (w ranges), in stage-1 consumption order
_XCHUNKS = [(124, 128), (0, 32), (32, 64), (64, 96), (96, 124)]


def _build_program():
    import concourse.mybir as mybir
    import concourse.tile as tile
    from concourse import bacc

    f32 = mybir.dt.float32
    bf16 = mybir.dt.bfloat16

    nc = bacc.Bacc("TRN2", target_bir_lowering=False, debug=False,
                   enable_asserts=False, num_devices=B)
    x_d = nc.dram_tensor("x", [H, W * 2 * CI], bf16, kind="ExternalInput").ap()
    g_d = nc.dram_tensor("g", [H, KH * H], bf16, kind="ExternalInput").ap()
    w_d = nc.dram_tensor("wt", [128, KW * NSLOT * CO], bf16,
                         kind="ExternalInput").ap()
    b_d = nc.dram_tensor("bias", [CO, 1], f32, kind="ExternalInput").ap()
    o_d = nc.dram_tensor("out", [CO, H, W], f32, kind="ExternalOutput").ap()

    with tile.TileContext(nc) as tc:
        with (
            tc.tile_pool(name="const", bufs=1) as cpool,
            tc.tile_pool(name="win", bufs=3) as wpool,
            tc.tile_pool(name="oacc", bufs=1) as opool,
            tc.tile_pool(name="ps1", bufs=4, space="PSUM") as ps1,
            tc.tile_pool(name="ps2", bufs=3, space="PSUM") as ps2,
        ):
            gt = cpool.tile([H, KH * H], bf16)
            nc.sync.dma_start(gt[:], g_d)
            # x in w-chunks so early stage-1 work doesn't wait on all of x
            xts = {}
            for (w0, w1) in _XCHUNKS:
                xt = cpool.tile([H, (w1 - w0) * 2 * CI], bf16,
                                tag=f"x{w0}")
                nc.sync.dma_start(xt[:], x_d[:, w0 * 2 * CI:w1 * 2 * CI])
                xts[(w0, w1)] = xt[:].rearrange("p (w di) -> p w di",
                                                di=2 * CI)
            wt = cpool.tile([128, KW * NSLOT * CO], bf16)
            nc.sync.dma_start(wt[:], w_d)
            bt = cpool.tile([CO, 1], f32)
            nc.sync.dma_start(bt[:], b_d)

            def xcol(w):
                for (w0, w1) in _XCHUNKS:
                    if w0 <= w < w1:
                        return xts[(w0, w1)][:, w - w0, :]
                raise AssertionError(w)

            # ---------------- emission units ----------------

            def s1_unit(mh, w, wtile, slot):
                """One stage-1 column: matmul + 2 contiguous psum->SBUF casts."""
                def emit():
                    p1 = ps1.tile([128, KH * MH], f32, name="p1")
                    nc.tensor.matmul(p1[:], xcol(w),
                                     gt[:, mh * KH * MH:(mh + 1) * KH * MH],
                                     start=True, stop=True)
                    w3 = wtile[:].rearrange("p (c j m) -> p c j m",
                                            c=NSLOT, j=WIN)
                    # top half keeps s={0,2,4} -> slots c=0..2 (contiguous 192)
                    nc.vector.tensor_copy(w3[0:64, :, slot, :],
                                          p1[0:64, 0:NSLOT * MH]
                                          .rearrange("p (c m) -> p c m", c=3))
                    # bottom half keeps s={1,3} -> slots c=0..1 (contiguous 128)
                    nc.scalar.copy(w3[64:128, 0:2, slot, :],
                                   p1[64:128, NSLOT * MH:KH * MH]
                                   .rearrange("p (c m) -> p c m", c=2))
                return emit

            def dup_unit(wtile, wtile_next):
                """Seed window k+1 slots 0..3 from window k slots 32..35
                (bf16 SBUF->SBUF on GpSimd; psum is off-limits there)."""
                def emit():
                    w3 = wtile[:].rearrange("p (c j m) -> p c j m",
                                            c=NSLOT, j=WIN)
                    n3 = wtile_next[:].rearrange("p (c j m) -> p c j m",
                                                 c=NSLOT, j=WIN)
                    nc.gpsimd.tensor_copy(n3[:, 0:2, 0:HALO, :],
                                          w3[:, 0:2, WB:WIN, :])
                    nc.gpsimd.tensor_copy(n3[0:64, 2, 0:HALO, :],
                                          w3[0:64, 2, WB:WIN, :])
                return emit

            def s2_mm_unit(wtile, blk, jt, t, c, p2holder):
                def emit():
                    if p2holder[0] is None:
                        p2holder[0] = ps2.tile([128, JT * MH], f32,
                                               name="p2")
                    p2 = p2holder[0]
                    kk = 128 if c < 2 else 64
                    lhsT2 = wt[0:kk, (t * NSLOT + c) * CO:
                               (t * NSLOT + c + 1) * CO]
                    w3 = wtile[:].rearrange("p (c j m) -> p c j m",
                                            c=NSLOT, j=WIN)
                    rhs2 = w3[0:kk, c, jt * JT - t + HALO:
                              (jt + 1) * JT - t + HALO, :]
                    start = (t == 0 and c == 0)
                    stop = (t == KW - 1 and c == NSLOT - 1)
                    nc.tensor.matmul(p2[:], lhsT2, rhs2,
                                     start=start, stop=stop)
                return emit

            def s2_evac_unit(mh, blk, jt, oa, p2holder):
                def emit():
                    p2 = p2holder[0]
                    p23 = p2[:].rearrange("p (j m) -> p j m", j=JT)
                    oa3 = oa[:].rearrange("p (m w) -> p m w", w=W)
                    dst = oa3[:, :, blk * WB + jt * JT:
                              blk * WB + (jt + 1) * JT]
                    src = p23[:].transpose([0, 2, 1])
                    if jt % 2 == 0:
                        nc.vector.tensor_scalar_add(dst, src, bt[:])
                    else:
                        nc.scalar.activation(
                            dst, src, mybir.ActivationFunctionType.Identity,
                            bias=bt[:])
                    p2holder[0] = None
                return emit

            # ---------------- schedule ----------------

            def s1_window_units(mh, k, wtile, wtile_next):
                units = []
                if k == 0:
                    fresh = list(range(124, 128)) + list(range(0, 32))
                else:
                    fresh = list(range(32 * k, 32 * k + 32))
                for w in fresh:
                    if k == 0 and w >= 124:
                        slot = w - 124
                    else:
                        slot = w - (32 * k - 4)
                    units.append(s1_unit(mh, w, wtile, slot))
                if k < NBLK - 1:
                    units.append(dup_unit(wtile, wtile_next))
                return units

            def s2_block_units(mh, k, wtile, oa):
                units = []
                for jt in range(WB // JT):
                    holder = [None]
                    for t in range(KW):
                        for c in range(NSLOT):
                            units.append(s2_mm_unit(wtile, k, jt, t, c,
                                                    holder))
                    units.append(s2_evac_unit(mh, k, jt, oa, holder))
                return units

            # build the full interleaved program
            oaccs = {}
            wtiles = {}

            def get_wtile(mh, k):
                if (mh, k) not in wtiles:
                    wtiles[(mh, k)] = wpool.tile(
                        [128, NSLOT * WIN * MH], bf16, name="win")
                return wtiles[(mh, k)]

            wins = [(mh, k) for mh in range(2) for k in range(NBLK)]
            for mh in range(2):
                oaccs[mh] = opool.tile([CO, MH * W], f32, tag=f"oacc{mh}",
                                       name=f"oacc{mh}")

            # prologue: first window fully
            mh0, k0 = wins[0]
            t0 = get_wtile(mh0, k0)
            t1 = get_wtile(*wins[1])
            for u in s1_window_units(mh0, k0, t0, t1):
                u()

            for idx, (mh, k) in enumerate(wins):
                wtile = get_wtile(mh, k)
                s2u = s2_block_units(mh, k, wtile, oaccs[mh])
                if idx + 1 < len(wins):
                    nmh, nk = wins[idx + 1]
                    ntile = get_wtile(nmh, nk)
                    nntile = (get_wtile(*wins[idx + 2])
                              if idx + 2 < len(wins) else None)
                    s1u = s1_window_units(nmh, nk, ntile, nntile)
                else:
                    s1u = []
                # interleave one s1 unit per two s2 units until exhausted
                si = 0
                for i, u in enumerate(s2u):
                    if i % 2 == 0 and si < len(s1u):
                        s1u[si]()
                        si += 1
                    u()
                for u in s1u[si:]:
                    u()
                if k == NBLK - 1:
                    nc.sync.dma_start(
                        o_d[:, mh * MH:(mh + 1) * MH, :],
                        oaccs[mh][:].rearrange("p (m w) -> p m w", w=W))
    nc.compile()
    return nc


def _get_prog():
    global _PROG
    if _PROG is None:
        _PROG = _build_program()
    return _PROG


def _build_wstack(weight):
    # wst[(d,i), (t, c, o)]: d=0 -> s=2c ; d=1 -> s=2c+1 (c<2), zeros for c=2
    wst = np.zeros((128, KW * NSLOT * CO), np.float32)
    for t in range(KW):
        for c in range(NSLOT):
            col = (t * NSLOT + c) * CO
            wst[0:64, col:col + CO] = weight[:, :, 2 * c, t].T
            if c < 2:
                wst[64:128, col:col + CO] = weight[:, :, 2 * c + 1, t].T
    return np.ascontiguousarray(wst).astype(_np_dt())


def kernel(x, weight, bias):
    from concourse.bass_utils import run_bass_kernel_spmd

    global _CONSTS
    if _CONSTS is None:
        _CONSTS = _build_consts()
    GT = _CONSTS

    x = np.ascontiguousarray(np.asarray(x, dtype=np.float32))
    weight = np.ascontiguousarray(np.asarray(weight, dtype=np.float32))
    bias = np.ascontiguousarray(np.asarray(bias, dtype=np.float32))

    wst = _build_wstack(weight)
    b2 = np.ascontiguousarray(bias.reshape(CO, 1))

    in_maps = []
    for b in range(B):
        # [h, (w, di)] with di = d*64 + i duplicated
        xt = x[b].transpose(1, 2, 0)                        # [H, W, ci]
        xdup = np.ascontiguousarray(
            np.concatenate([xt, xt], axis=2).reshape(H, W * 2 * CI)
        ).astype(_np_dt())
        in_maps.append({"x": xdup, "g": GT, "wt": wst, "bias": b2})

    res = run_bass_kernel_spmd(_get_prog(), in_maps, core_ids=list(range(B)),
                               **_RUN_OPTS)
    global _LAST_RESULT
    _LAST_RESULT = res
    out = np.stack([res.results[b]["out"] for b in range(B)], axis=0)
    return np.ascontiguousarray(out.astype(np.float32))
